# revision 3
# baseline (speedup 1.0000x reference)
"""Cross-modal attention (CMAttention) Trainium2 kernel.

Strategy: 8-way SPMD over (batch=4) x (modality=2). After the reference's
concat([q_x, q_a]) and 8-head split with head_dim=128, heads 0-3 depend only
on modality x and heads 4-7 only on modality a.  Each core therefore owns one
(batch, modality) pair end-to-end with zero communication:
  QKV projection (bf16 matmul) -> LayerNorm on q,k (bn_stats) -> RoPE
  (table multiplies) -> per-head DMA-transpose of q,k to [d, tok] ->
  scores^T matmul -> exp on ScalarE (scale folded) -> attn @ [v | 1]
  (ones column yields the softmax denominator for free) -> normalize.
"""

import os
import sys

for _p in ("/opt/trn_rl_repo", os.path.expanduser("~/.axon_site/_ro/trn_rl_repo")):
    if os.path.isdir(_p) and _p not in sys.path:
        sys.path.append(_p)

from contextlib import ExitStack

import ml_dtypes
import numpy as np

import concourse.bacc as bacc
import concourse.bass as bass
import concourse.mybir as mybir
import concourse.tile as tile
from concourse.bass_utils import run_bass_kernel_spmd

BF16 = mybir.dt.bfloat16
F32 = mybir.dt.float32
NPBF16 = ml_dtypes.bfloat16

DIM = 512          # per-modality feature dim
N_TOK = 1024       # sequence length
NH = 4             # heads handled per core (one modality's heads)
D = 128            # head dim
NT = 8             # token tiles of 128
EPS = 1e-5
SCALE = 1.0 / float(np.sqrt(D))
VW = 132           # per-head v block width: 128 d + 1 ones + 3 pad


def _load_tiled(nc, sbuf_tile, dram, blocks):
    """DMA a [blocks*128, C] DRAM tensor into a [128, blocks, C] SBUF tile."""
    nc.sync.dma_start(out=sbuf_tile, in_=dram.ap().rearrange("(a b) c -> b a c", b=128))


def build_module(trivial: bool):
    """Build the per-core Bass program.  trivial=True assumes all LN gains are
    exactly 1 and biases exactly 0 (folded tables are plain cos/sin and the
    additive rope term vanishes); trivial=False uses full-width tables with
    g folded in and an extra additive T3 table."""
    nc = bacc.Bacc("TRN2", target_bir_lowering=False, debug=False, num_devices=8)

    xT = nc.dram_tensor("xT", [DIM, N_TOK], BF16, kind="ExternalInput")
    W = nc.dram_tensor("W", [DIM, 3 * DIM], BF16, kind="ExternalInput")
    if trivial:
        T1 = nc.dram_tensor("T1", [N_TOK, 64], BF16, kind="ExternalInput")
        T2N = nc.dram_tensor("T2N", [N_TOK, 64], BF16, kind="ExternalInput")
        T2P = nc.dram_tensor("T2P", [N_TOK, 64], BF16, kind="ExternalInput")
    else:
        T1 = nc.dram_tensor("T1", [N_TOK, 1024], BF16, kind="ExternalInput")
        T2 = nc.dram_tensor("T2", [N_TOK, 1024], BF16, kind="ExternalInput")
        T3 = nc.dram_tensor("T3", [N_TOK, 1024], BF16, kind="ExternalInput")
    out_d = nc.dram_tensor("out", [N_TOK, DIM], F32, kind="ExternalOutput")

    with tile.TileContext(nc) as tc, ExitStack() as ctx:
        consts = ctx.enter_context(tc.tile_pool(name="consts", bufs=1))
        small = ctx.enter_context(tc.tile_pool(name="small", bufs=4))
        upool = ctx.enter_context(tc.tile_pool(name="upool", bufs=2))
        rpool = ctx.enter_context(tc.tile_pool(name="rpool", bufs=2))
        epool = ctx.enter_context(tc.tile_pool(name="epool", bufs=2))
        psum_big = ctx.enter_context(tc.tile_pool(name="psb", bufs=2, space="PSUM"))
        psum_av = ctx.enter_context(tc.tile_pool(name="psa", bufs=2, space="PSUM"))

        # ---- constants ----
        xT_sb = consts.tile([128, 4, N_TOK], BF16, tag="xT")
        _load_tiled(nc, xT_sb, xT, 4)
        W_sb = consts.tile([128, 4, 3 * DIM], BF16, tag="W")
        _load_tiled(nc, W_sb, W, 4)
        if trivial:
            cos_sb = consts.tile([128, NT, 64], BF16, tag="cos")
            _load_tiled(nc, cos_sb, T1, NT)
            sinN_sb = consts.tile([128, NT, 64], BF16, tag="sinN")
            _load_tiled(nc, sinN_sb, T2N, NT)
            sinP_sb = consts.tile([128, NT, 64], BF16, tag="sinP")
            _load_tiled(nc, sinP_sb, T2P, NT)
        else:
            T1_sb = consts.tile([128, NT, 1024], BF16, tag="T1")
            _load_tiled(nc, T1_sb, T1, NT)
            T2_sb = consts.tile([128, NT, 1024], BF16, tag="T2")
            _load_tiled(nc, T2_sb, T2, NT)
            T3_sb = consts.tile([128, NT, 1024], BF16, tag="T3")
            _load_tiled(nc, T3_sb, T3, NT)
        eps_sb = consts.tile([128, 1], F32, tag="eps")
        nc.vector.memset(eps_sb, EPS)

        v_sb = consts.tile([128, NT, NH, VW], BF16, tag="v")
        nc.vector.memset(v_sb[:, :, :, 128:129], 1.0)

        qkT_sb = [
            [
                consts.tile(
                    [128, N_TOK], BF16, name=f"qkT{s}{h}", tag=f"qkT{s}{h}"
                )
                for h in range(NH)
            ]
            for s in range(2)
        ]
        out_sb = consts.tile([128, NT, DIM], F32, tag="osb")

        def bcast(ap2d, dims):
            """[128, 64] AP -> [128, *dims, 64] with stride-0 broadcast dims."""
            p, last = ap2d.ap[0], ap2d.ap[-1]
            return bass.AP(
                tensor=ap2d.tensor,
                offset=ap2d.offset,
                ap=[p] + [[0, d] for d in dims] + [last],
            )

        # ---- stage A: qkv, layernorm, rope, transposes ----
        for t in range(NT):
            qkv_ps = psum_big.tile([128, 3 * DIM], F32, tag="big")
            for kc in range(4):
                for j in range(3):
                    nc.tensor.matmul(
                        qkv_ps[:, j * 512 : (j + 1) * 512],
                        lhsT=xT_sb[:, kc, t * 128 : (t + 1) * 128],
                        rhs=W_sb[:, kc, j * 512 : (j + 1) * 512],
                        start=(kc == 0),
                        stop=(kc == 3),
                    )

            # u: normalized q|k, bf16, flat [part, (s h half j) = 1024]
            u = upool.tile([128, 2 * DIM], BF16, tag="u")
            for s in range(2):
                st = small.tile([128, 6], F32, tag=f"st{s}")
                nc.vector.bn_stats(out=st, in_=qkv_ps[:, s * 512 : (s + 1) * 512])
                mv = small.tile([128, 2], F32, tag=f"mv{s}")
                nc.vector.bn_aggr(out=mv, in_=st)
                negmu = small.tile([128, 1], F32, tag=f"nm{s}")
                nc.vector.tensor_scalar_mul(negmu, mv[:, 0:1], -1.0)
                sd = small.tile([128, 1], F32, tag=f"sd{s}")
                nc.scalar.activation(
                    sd, mv[:, 1:2], mybir.ActivationFunctionType.Sqrt, bias=eps_sb
                )
                rstd = small.tile([128, 1], F32, tag=f"rs{s}")
                nc.vector.reciprocal(rstd, sd)
                nc.vector.tensor_scalar(
                    out=u[:, s * 512 : (s + 1) * 512],
                    in0=qkv_ps[:, s * 512 : (s + 1) * 512],
                    scalar1=negmu,
                    scalar2=rstd,
                    op0=mybir.AluOpType.add,
                    op1=mybir.AluOpType.mult,
                )

            # v (raw) into augmented per-head layout
            nc.vector.tensor_copy(
                out=v_sb[:, t, :, 0:128],
                in_=qkv_ps[:, 1024:1536].rearrange("p (h d) -> p h d", h=NH),
            )

            # rope: r = u * T1 + swap_half(u) * T2 (+ T3).
            # half(ap, i): [128, 8 blocks, 64] view selecting half i of each
            # (s, h) 128-wide block -- max 3D free APs for the ISA.
            def half(ap, i):
                return ap.rearrange("p (b half j) -> p b half j", half=2, j=64)[
                    :, :, i, :
                ]

            m1 = rpool.tile([128, 2 * DIM], BF16, tag="m1")
            m2 = rpool.tile([128, 2 * DIM], BF16, tag="m2")
            r = rpool.tile([128, 2 * DIM], BF16, tag="r")
            if trivial:
                nc.vector.tensor_mul(
                    m1.rearrange("p (b j) -> p b j", j=64),
                    u.rearrange("p (b j) -> p b j", j=64),
                    bcast(cos_sb[:, t], (16,)),
                )
                nc.vector.tensor_mul(half(m2, 0), half(u, 1), bcast(sinN_sb[:, t], (8,)))
                nc.vector.tensor_mul(half(m2, 1), half(u, 0), bcast(sinP_sb[:, t], (8,)))
                nc.vector.tensor_add(r, m1, m2)
            else:
                t1v = T1_sb[:, t]
                t2v = T2_sb[:, t]
                t3v = T3_sb[:, t]
                nc.vector.tensor_mul(m1, u, t1v)
                nc.vector.tensor_mul(half(m2, 0), half(u, 1), half(t2v, 0))
                nc.vector.tensor_mul(half(m2, 1), half(u, 0), half(t2v, 1))
                nc.vector.tensor_add(m1, m1, m2)
                nc.vector.tensor_add(r, m1, t3v)

            # per-(s, head) 128x128 DMA transposes into [d, tok] layout
            for s in range(2):
                for h in range(NH):
                    nc.sync.dma_start(
                        out=qkT_sb[s][h][:, t * 128 : (t + 1) * 128],
                        in_=r[:, (s * NH + h) * 128 : (s * NH + h + 1) * 128],
                        transpose=True,
                    )

        # ---- stage B: per-head attention ----
        for h in range(NH):
            qT, kT = qkT_sb[0][h], qkT_sb[1][h]
            expts = []
            for kc in range(NT):
                sc_ps = psum_big.tile([128, 3 * DIM], F32, tag="big")
                for half in range(2):
                    nc.tensor.matmul(
                        sc_ps[:, half * 512 : (half + 1) * 512],
                        lhsT=kT[:, kc * 128 : (kc + 1) * 128],
                        rhs=qT[:, half * 512 : (half + 1) * 512],
                        start=True,
                        stop=True,
                    )
                et = epool.tile([128, N_TOK], BF16, tag=f"exp{kc}")
                nc.scalar.activation(
                    out=et,
                    in_=sc_ps[:, 0:N_TOK],
                    func=mybir.ActivationFunctionType.Exp,
                    scale=SCALE,
                )
                expts.append(et)
            for qc in range(NT):
                av = psum_av.tile([128, VW], F32, tag="av")
                for kc in range(NT):
                    nc.tensor.matmul(
                        av[:, 0:129],
                        lhsT=expts[kc][:, qc * 128 : (qc + 1) * 128],
                        rhs=v_sb[:, kc, h, 0:129],
                        start=(kc == 0),
                        stop=(kc == NT - 1),
                    )
                rcp = small.tile([128, 1], F32, tag="rcp")
                nc.vector.reciprocal(rcp, av[:, 128:129])
                nc.vector.tensor_scalar_mul(
                    out_sb[:, qc, h * 128 : (h + 1) * 128], av[:, 0:128], rcp
                )
                if h == NH - 1:
                    nc.sync.dma_start(
                        out=out_d.ap()[qc * 128 : (qc + 1) * 128, :],
                        in_=out_sb[:, qc],
                    )

    nc.compile()
    return nc


def _rope_tables():
    inv_freq = 1.0 / (10000.0 ** (np.arange(0, D, 2, dtype=np.float32) / D))
    freqs = np.arange(N_TOK, dtype=np.float32)[:, None] * inv_freq[None, :]  # [n, 64]
    return np.cos(freqs), np.sin(freqs)


def _full_tables(g_q, b_q, g_k, b_k):
    """T1/T2/T3 [N_TOK, 1024] with LN gain/bias folded into the rope tables.
    Feature index layout matches u: (s, h, half, j)."""
    cos64, sin64 = _rope_tables()
    T1 = np.empty((N_TOK, 1024), np.float32)
    T2 = np.empty((N_TOK, 1024), np.float32)
    T3 = np.empty((N_TOK, 1024), np.float32)
    for s, (g, b) in enumerate(((g_q, b_q), (g_k, b_k))):
        g = g.reshape(NH, 2, 64)
        b = b.reshape(NH, 2, 64)
        for h in range(NH):
            base = s * 512 + h * 128
            lo, hi = slice(base, base + 64), slice(base + 64, base + 128)
            T1[:, lo] = g[h, 0] * cos64
            T1[:, hi] = g[h, 1] * cos64
            T2[:, lo] = -g[h, 1] * sin64
            T2[:, hi] = g[h, 0] * sin64
            T3[:, lo] = b[h, 0] * cos64 - b[h, 1] * sin64
            T3[:, hi] = b[h, 1] * cos64 + b[h, 0] * sin64
    return T1, T2, T3


def make_in_maps(x, a, Wqkv_x, Wqkv_a, g_qx, b_qx, g_kx, b_kx, g_qa, b_qa, g_ka, b_ka):
    """Returns (trivial, in_maps) for the 8 cores: core c = (batch c//2, modality c%2)."""
    x, a = np.asarray(x), np.asarray(a)
    Ws = (np.asarray(Wqkv_x), np.asarray(Wqkv_a))
    gb = (
        (np.asarray(g_qx), np.asarray(b_qx), np.asarray(g_kx), np.asarray(b_kx)),
        (np.asarray(g_qa), np.asarray(b_qa), np.asarray(g_ka), np.asarray(b_ka)),
    )
    trivial = all(
        np.all(g == 1.0) and np.all(b == 0.0)
        for (gq, bq, gk, bk) in gb
        for g, b in ((gq, bq), (gk, bk))
    )
    cos64, sin64 = _rope_tables()
    in_maps = []
    for c in range(8):
        i, m = c // 2, c % 2
        src = x[i] if m == 0 else a[i]
        im = {
            "xT": np.ascontiguousarray(src.T).astype(NPBF16),
            "W": Ws[m].astype(NPBF16),
        }
        if trivial:
            im["T1"] = cos64.astype(NPBF16)
            im["T2N"] = (-sin64).astype(NPBF16)
            im["T2P"] = sin64.astype(NPBF16)
        else:
            gq, bq, gk, bk = gb[m]
            T1, T2, T3 = _full_tables(gq, bq, gk, bk)
            im["T1"] = T1.astype(NPBF16)
            im["T2"] = T2.astype(NPBF16)
            im["T3"] = T3.astype(NPBF16)
        in_maps.append(im)
    return trivial, in_maps


_module_cache: dict[bool, object] = {}


def _get_module(trivial: bool):
    if trivial not in _module_cache:
        _module_cache[trivial] = build_module(trivial)
    return _module_cache[trivial]


def kernel(**inputs) -> np.ndarray:
    trivial, in_maps = make_in_maps(**inputs)
    nc = _get_module(trivial)
    res = run_bass_kernel_spmd(nc, in_maps, core_ids=list(range(8)))
    out = np.empty((4, N_TOK, 2 * DIM), np.float32)
    for c in range(8):
        i, m = c // 2, c % 2
        out[i, :, m * 512 : (m + 1) * 512] = res.results[c]["out"]
    return out


# revision 8
# speedup vs baseline: 1.1314x; 1.1314x over previous
"""Cross-modal attention (CMAttention) Trainium2 kernel.

Strategy: 8-way SPMD over (batch=4) x (modality=2). After the reference's
concat([q_x, q_a]) and 8-head split with head_dim=128, heads 0-3 depend only
on modality x and heads 4-7 only on modality a.  Each core therefore owns one
(batch, modality) pair end-to-end with zero communication:
  QKV projection (bf16 matmul) -> LayerNorm on q,k (bn_stats) -> RoPE
  (table multiplies) -> per-head DMA-transpose of q,k to [d, tok] ->
  scores^T matmul -> exp on ScalarE (scale folded) -> attn @ [v | 1]
  (ones column yields the softmax denominator for free) -> normalize.
"""

import os
import sys

for _p in ("/opt/trn_rl_repo", os.path.expanduser("~/.axon_site/_ro/trn_rl_repo")):
    if os.path.isdir(_p) and _p not in sys.path:
        sys.path.append(_p)

from contextlib import ExitStack

import ml_dtypes
import numpy as np

import concourse.bacc as bacc
import concourse.bass as bass
import concourse.mybir as mybir
import concourse.tile as tile
from concourse.bass_utils import run_bass_kernel_spmd

BF16 = mybir.dt.bfloat16
F32 = mybir.dt.float32
NPBF16 = ml_dtypes.bfloat16

DIM = 512          # per-modality feature dim
N_TOK = 1024       # sequence length
NH = 4             # heads handled per core (one modality's heads)
D = 128            # head dim
NT = 8             # token tiles of 128
EPS = 1e-5
SCALE = 1.0 / float(np.sqrt(D))
VW = 132           # per-head v block width: 128 d + 1 ones + 3 pad


def _load_tiled(nc, sbuf_tile, dram, blocks):
    """DMA a [blocks*128, C] DRAM tensor into a [128, blocks, C] SBUF tile."""
    nc.sync.dma_start(out=sbuf_tile, in_=dram.ap().rearrange("(a b) c -> b a c", b=128))


def build_module(trivial: bool):
    """Build the per-core Bass program.  trivial=True assumes all LN gains are
    exactly 1 and biases exactly 0 (folded tables are plain cos/sin and the
    additive rope term vanishes); trivial=False uses full-width tables with
    g folded in and an extra additive T3 table."""
    nc = bacc.Bacc("TRN2", target_bir_lowering=False, debug=False, num_devices=8)

    xT = nc.dram_tensor("xT", [DIM, N_TOK], BF16, kind="ExternalInput")
    W = nc.dram_tensor("W", [DIM, 3 * DIM], BF16, kind="ExternalInput")
    if trivial:
        T1 = nc.dram_tensor("T1", [N_TOK, 64], BF16, kind="ExternalInput")
        T2N = nc.dram_tensor("T2N", [N_TOK, 64], BF16, kind="ExternalInput")
        T2P = nc.dram_tensor("T2P", [N_TOK, 64], BF16, kind="ExternalInput")
    else:
        T1 = nc.dram_tensor("T1", [N_TOK, 1024], BF16, kind="ExternalInput")
        T2 = nc.dram_tensor("T2", [N_TOK, 1024], BF16, kind="ExternalInput")
        T3 = nc.dram_tensor("T3", [N_TOK, 1024], BF16, kind="ExternalInput")
    out_d = nc.dram_tensor("out", [N_TOK, DIM], F32, kind="ExternalOutput")

    with tile.TileContext(nc) as tc, ExitStack() as ctx:
        consts = ctx.enter_context(tc.tile_pool(name="consts", bufs=1))
        small = ctx.enter_context(tc.tile_pool(name="small", bufs=4))
        upool = ctx.enter_context(tc.tile_pool(name="upool", bufs=2))
        rpool = ctx.enter_context(tc.tile_pool(name="rpool", bufs=2))
        epool = ctx.enter_context(tc.tile_pool(name="epool", bufs=2))
        dpool = ctx.enter_context(tc.tile_pool(name="dpool", bufs=1, space="DRAM"))
        psum_big = ctx.enter_context(tc.tile_pool(name="psb", bufs=2, space="PSUM"))
        psum_av = ctx.enter_context(tc.tile_pool(name="psa", bufs=2, space="PSUM"))

        # ---- constants ----
        xT_sb = consts.tile([128, 4, N_TOK], BF16, tag="xT")
        _load_tiled(nc, xT_sb, xT, 4)
        W_sb = consts.tile([128, 4, 3 * DIM], BF16, tag="W")
        _load_tiled(nc, W_sb, W, 4)
        if trivial:
            cos_sb = consts.tile([128, NT, 64], BF16, tag="cos")
            _load_tiled(nc, cos_sb, T1, NT)
            sinN_sb = consts.tile([128, NT, 64], BF16, tag="sinN")
            _load_tiled(nc, sinN_sb, T2N, NT)
            sinP_sb = consts.tile([128, NT, 64], BF16, tag="sinP")
            _load_tiled(nc, sinP_sb, T2P, NT)
        else:
            T1_sb = consts.tile([128, NT, 1024], BF16, tag="T1")
            _load_tiled(nc, T1_sb, T1, NT)
            T2_sb = consts.tile([128, NT, 1024], BF16, tag="T2")
            _load_tiled(nc, T2_sb, T2, NT)
            T3_sb = consts.tile([128, NT, 1024], BF16, tag="T3")
            _load_tiled(nc, T3_sb, T3, NT)
        eps_sb = consts.tile([128, 1], F32, tag="eps")
        nc.vector.memset(eps_sb, EPS)

        v_sb = consts.tile([128, NT, NH, VW], BF16, tag="v")
        nc.vector.memset(v_sb[:, :, :, 128:129], 1.0)

        qkT_sb = [
            [
                consts.tile(
                    [128, N_TOK], BF16, name=f"qkT{s}{h}", tag=f"qkT{s}{h}"
                )
                for h in range(NH)
            ]
            for s in range(2)
        ]
        out_sb = consts.tile([128, NT, DIM], F32, tag="osb")
        r_dram = dpool.tile([N_TOK, 2 * DIM], BF16, name="r_dram", tag="r_dram")

        def bcast(ap2d, dims):
            """[128, 64] AP -> [128, *dims, 64] with stride-0 broadcast dims."""
            p, last = ap2d.ap[0], ap2d.ap[-1]
            return bass.AP(
                tensor=ap2d.tensor,
                offset=ap2d.offset,
                ap=[p] + [[0, d] for d in dims] + [last],
            )

        # ---- stage A: qkv, layernorm, rope, transposes ----
        for t in range(NT):
            qkv_ps = psum_big.tile([128, 3 * DIM], F32, tag="big")
            for kc in range(4):
                for j in range(3):
                    nc.tensor.matmul(
                        qkv_ps[:, j * 512 : (j + 1) * 512],
                        lhsT=xT_sb[:, kc, t * 128 : (t + 1) * 128],
                        rhs=W_sb[:, kc, j * 512 : (j + 1) * 512],
                        start=(kc == 0),
                        stop=(kc == 3),
                    )

            # u: normalized q|k, bf16, flat [part, (s h half j) = 1024]
            u = upool.tile([128, 2 * DIM], BF16, tag="u")
            for s in range(2):
                st = small.tile([128, 6], F32, tag=f"st{s}")
                nc.vector.bn_stats(out=st, in_=qkv_ps[:, s * 512 : (s + 1) * 512])
                mv = small.tile([128, 2], F32, tag=f"mv{s}")
                nc.vector.bn_aggr(out=mv, in_=st)
                sd = small.tile([128, 1], F32, tag=f"sd{s}")
                nc.scalar.activation(
                    sd, mv[:, 1:2], mybir.ActivationFunctionType.Sqrt, bias=eps_sb
                )
                rstd = small.tile([128, 1], F32, tag=f"rs{s}")
                nc.vector.reciprocal(rstd, sd)
                nmr = small.tile([128, 1], F32, tag=f"nmr{s}")
                nc.vector.scalar_tensor_tensor(
                    out=nmr,
                    in0=mv[:, 0:1],
                    scalar=-1.0,
                    in1=rstd,
                    op0=mybir.AluOpType.mult,
                    op1=mybir.AluOpType.mult,
                )
                # u = q * rstd + (-mu * rstd), on ScalarE (frees VectorE)
                nc.scalar.activation(
                    out=u[:, s * 512 : (s + 1) * 512],
                    in_=qkv_ps[:, s * 512 : (s + 1) * 512],
                    func=mybir.ActivationFunctionType.Identity,
                    scale=rstd,
                    bias=nmr,
                )

            # v (raw) into augmented per-head layout
            nc.vector.tensor_copy(
                out=v_sb[:, t, :, 0:128],
                in_=qkv_ps[:, 1024:1536].rearrange("p (h d) -> p h d", h=NH),
            )

            # rope: r = u * T1 + swap_half(u) * T2 (+ T3).
            # half(ap, i): [128, 8 blocks, 64] view selecting half i of each
            # (s, h) 128-wide block -- max 3D free APs for the ISA.
            def half(ap, i):
                return ap.rearrange("p (b half j) -> p b half j", half=2, j=64)[
                    :, :, i, :
                ]

            m1 = rpool.tile([128, 2 * DIM], BF16, tag="m1")
            m2 = rpool.tile([128, 2 * DIM], BF16, tag="m2")
            r = rpool.tile([128, 2 * DIM], BF16, tag="r")
            if trivial:
                nc.gpsimd.tensor_mul(
                    m1.rearrange("p (b j) -> p b j", j=64),
                    u.rearrange("p (b j) -> p b j", j=64),
                    bcast(cos_sb[:, t], (16,)),
                )
                nc.gpsimd.tensor_mul(half(m2, 0), half(u, 1), bcast(sinN_sb[:, t], (8,)))
                nc.gpsimd.tensor_mul(half(m2, 1), half(u, 0), bcast(sinP_sb[:, t], (8,)))
                nc.vector.tensor_add(r, m1, m2)
            else:
                t1v = T1_sb[:, t]
                t2v = T2_sb[:, t]
                t3v = T3_sb[:, t]
                nc.gpsimd.tensor_mul(m1, u, t1v)
                nc.gpsimd.tensor_mul(half(m2, 0), half(u, 1), half(t2v, 0))
                nc.gpsimd.tensor_mul(half(m2, 1), half(u, 0), half(t2v, 1))
                nc.vector.tensor_add(m1, m1, m2)
                nc.vector.tensor_add(r, m1, t3v)

            # spill rope output to DRAM; bulk feature-major transposes later
            nc.sync.dma_start(out=r_dram[t * 128 : (t + 1) * 128, :], in_=r)

        # big DRAM->SBUF transposes: [1024 tok, 128 feat] -> [128 d, 1024 tok]
        for s in range(2):
            for h in range(NH):
                blk = (s * NH + h) * 128
                nc.sync.dma_start(
                    out=qkT_sb[s][h],
                    in_=r_dram[:, blk : blk + 128],
                    transpose=True,
                )

        # ---- stage B: per-head attention ----
        def emit_scores(h):
            qT, kT = qkT_sb[0][h], qkT_sb[1][h]
            expts = []
            for kc in range(NT):
                sc_ps = psum_big.tile([128, 3 * DIM], F32, tag="big", name="sc_ps")
                for half in range(2):
                    nc.tensor.matmul(
                        sc_ps[:, half * 512 : (half + 1) * 512],
                        lhsT=kT[:, kc * 128 : (kc + 1) * 128],
                        rhs=qT[:, half * 512 : (half + 1) * 512],
                        start=True,
                        stop=True,
                    )
                et = epool.tile([128, N_TOK], BF16, tag=f"exp{kc}", name=f"exp{kc}")
                nc.scalar.activation(
                    out=et,
                    in_=sc_ps[:, 0:N_TOK],
                    func=mybir.ActivationFunctionType.Exp,
                    scale=SCALE,
                )
                expts.append(et)
            return expts

        def emit_av(h, expts):
            for qc in range(NT):
                av = psum_av.tile([128, VW], F32, tag="av", name="av")
                for kc in range(NT):
                    nc.tensor.matmul(
                        av[:, 0:129],
                        lhsT=expts[kc][:, qc * 128 : (qc + 1) * 128],
                        rhs=v_sb[:, kc, h, 0:129],
                        start=(kc == 0),
                        stop=(kc == NT - 1),
                    )
                rcp = small.tile([128, 1], F32, tag="rcp", name="rcp")
                nc.vector.reciprocal(rcp, av[:, 128:129])
                nc.vector.tensor_scalar_mul(
                    out_sb[:, qc, h * 128 : (h + 1) * 128], av[:, 0:128], rcp
                )
                if h == NH - 1:
                    nc.sync.dma_start(
                        out=out_d.ap()[qc * 128 : (qc + 1) * 128, :],
                        in_=out_sb[:, qc],
                    )

        # interleave: emit scores(h+1) before av(h) so PE fills exp-wait gaps
        exp_cur = emit_scores(0)
        for h in range(NH):
            exp_next = emit_scores(h + 1) if h + 1 < NH else None
            emit_av(h, exp_cur)
            exp_cur = exp_next

    nc.compile()
    return nc


def _rope_tables():
    inv_freq = 1.0 / (10000.0 ** (np.arange(0, D, 2, dtype=np.float32) / D))
    freqs = np.arange(N_TOK, dtype=np.float32)[:, None] * inv_freq[None, :]  # [n, 64]
    return np.cos(freqs), np.sin(freqs)


def _full_tables(g_q, b_q, g_k, b_k):
    """T1/T2/T3 [N_TOK, 1024] with LN gain/bias folded into the rope tables.
    Feature index layout matches u: (s, h, half, j)."""
    cos64, sin64 = _rope_tables()
    T1 = np.empty((N_TOK, 1024), np.float32)
    T2 = np.empty((N_TOK, 1024), np.float32)
    T3 = np.empty((N_TOK, 1024), np.float32)
    for s, (g, b) in enumerate(((g_q, b_q), (g_k, b_k))):
        g = g.reshape(NH, 2, 64)
        b = b.reshape(NH, 2, 64)
        for h in range(NH):
            base = s * 512 + h * 128
            lo, hi = slice(base, base + 64), slice(base + 64, base + 128)
            T1[:, lo] = g[h, 0] * cos64
            T1[:, hi] = g[h, 1] * cos64
            T2[:, lo] = -g[h, 1] * sin64
            T2[:, hi] = g[h, 0] * sin64
            T3[:, lo] = b[h, 0] * cos64 - b[h, 1] * sin64
            T3[:, hi] = b[h, 1] * cos64 + b[h, 0] * sin64
    return T1, T2, T3


def make_in_maps(x, a, Wqkv_x, Wqkv_a, g_qx, b_qx, g_kx, b_kx, g_qa, b_qa, g_ka, b_ka):
    """Returns (trivial, in_maps) for the 8 cores: core c = (batch c//2, modality c%2)."""
    x, a = np.asarray(x), np.asarray(a)
    Ws = (np.asarray(Wqkv_x), np.asarray(Wqkv_a))
    gb = (
        (np.asarray(g_qx), np.asarray(b_qx), np.asarray(g_kx), np.asarray(b_kx)),
        (np.asarray(g_qa), np.asarray(b_qa), np.asarray(g_ka), np.asarray(b_ka)),
    )
    trivial = all(
        np.all(g == 1.0) and np.all(b == 0.0)
        for (gq, bq, gk, bk) in gb
        for g, b in ((gq, bq), (gk, bk))
    )
    cos64, sin64 = _rope_tables()
    in_maps = []
    for c in range(8):
        i, m = c // 2, c % 2
        src = x[i] if m == 0 else a[i]
        im = {
            "xT": np.ascontiguousarray(src.T).astype(NPBF16),
            "W": Ws[m].astype(NPBF16),
        }
        if trivial:
            im["T1"] = cos64.astype(NPBF16)
            im["T2N"] = (-sin64).astype(NPBF16)
            im["T2P"] = sin64.astype(NPBF16)
        else:
            gq, bq, gk, bk = gb[m]
            T1, T2, T3 = _full_tables(gq, bq, gk, bk)
            im["T1"] = T1.astype(NPBF16)
            im["T2"] = T2.astype(NPBF16)
            im["T3"] = T3.astype(NPBF16)
        in_maps.append(im)
    return trivial, in_maps


_module_cache: dict[bool, object] = {}


def _get_module(trivial: bool):
    if trivial not in _module_cache:
        _module_cache[trivial] = build_module(trivial)
    return _module_cache[trivial]


def kernel(**inputs) -> np.ndarray:
    trivial, in_maps = make_in_maps(**inputs)
    nc = _get_module(trivial)
    res = run_bass_kernel_spmd(nc, in_maps, core_ids=list(range(8)))
    out = np.empty((4, N_TOK, 2 * DIM), np.float32)
    for c in range(8):
        i, m = c // 2, c % 2
        out[i, :, m * 512 : (m + 1) * 512] = res.results[c]["out"]
    return out


# revision 15
# speedup vs baseline: 1.3992x; 1.2367x over previous
"""Cross-modal attention (CMAttention) Trainium2 kernel.

Strategy: 8-way SPMD over (batch=4) x (modality=2). After the reference's
concat([q_x, q_a]) and 8-head split with head_dim=128, heads 0-3 depend only
on modality x and heads 4-7 only on modality a.  Each core therefore owns one
(batch, modality) pair end-to-end with zero communication:
  QKV projection (bf16 matmul) -> LayerNorm on q,k (bn_stats) -> RoPE
  (table multiplies) -> per-head DMA-transpose of q,k to [d, tok] ->
  scores^T matmul -> exp on ScalarE (scale folded) -> attn @ [v | 1]
  (ones column yields the softmax denominator for free) -> normalize.
"""

import os
import sys

for _p in ("/opt/trn_rl_repo", os.path.expanduser("~/.axon_site/_ro/trn_rl_repo")):
    if os.path.isdir(_p) and _p not in sys.path:
        sys.path.append(_p)

from contextlib import ExitStack

import ml_dtypes
import numpy as np

import concourse.bacc as bacc
import concourse.bass as bass
import concourse.mybir as mybir
import concourse.tile as tile
from concourse.bass_utils import run_bass_kernel_spmd

if os.environ.get("K_LDWOPT"):
    import concourse.bass_utils as _bu

    _orig_run_command = _bu.run_command

    def _patched_run_command(argv, **kw):
        argv = [
            "--enable-ldw-opt=true" if a == "--enable-ldw-opt=false" else a
            for a in argv
        ]
        return _orig_run_command(argv, **kw)

    _bu.run_command = _patched_run_command

BF16 = mybir.dt.bfloat16
F32 = mybir.dt.float32
NPBF16 = ml_dtypes.bfloat16

DIM = 512          # per-modality feature dim
N_TOK = 1024       # sequence length
NH = 4             # heads handled per core (one modality's heads)
D = 128            # head dim
NT = 8             # token tiles of 128
EPS = 1e-5
SCALE = 1.0 / float(np.sqrt(D))
VW = 132           # per-head v block width: 128 d + 1 ones + 3 pad


def _load_tiled(nc, sbuf_tile, dram, blocks):
    """DMA a [blocks*128, C] DRAM tensor into a [128, blocks, C] SBUF tile.
    GpSimd's SWDGE ring is idle at kernel start; using it keeps the sync
    ring free for the xT/W loads that gate the first matmul."""
    nc.gpsimd.dma_start(
        out=sbuf_tile, in_=dram.ap().rearrange("(a b) c -> b a c", b=128)
    )


def build_module(trivial: bool):
    """Build the per-core Bass program.  trivial=True assumes all LN gains are
    exactly 1 and biases exactly 0 (folded tables are plain cos/sin and the
    additive rope term vanishes); trivial=False uses full-width tables with
    g folded in and an extra additive T3 table."""
    nc = bacc.Bacc("TRN2", target_bir_lowering=False, debug=False, num_devices=8)

    xT = nc.dram_tensor("xT", [DIM, N_TOK], BF16, kind="ExternalInput")
    W = nc.dram_tensor("W", [DIM, 3 * DIM], BF16, kind="ExternalInput")
    if trivial:
        T1 = nc.dram_tensor("T1", [N_TOK, 64], BF16, kind="ExternalInput")
        T2N = nc.dram_tensor("T2N", [N_TOK, 64], BF16, kind="ExternalInput")
        T2P = nc.dram_tensor("T2P", [N_TOK, 64], BF16, kind="ExternalInput")
    else:
        T1 = nc.dram_tensor("T1", [N_TOK, 1024], BF16, kind="ExternalInput")
        T2 = nc.dram_tensor("T2", [N_TOK, 1024], BF16, kind="ExternalInput")
        T3 = nc.dram_tensor("T3", [N_TOK, 1024], BF16, kind="ExternalInput")
    out_d = nc.dram_tensor("out", [N_TOK, DIM], F32, kind="ExternalOutput")

    with tile.TileContext(nc) as tc, ExitStack() as ctx:
        consts = ctx.enter_context(tc.tile_pool(name="consts", bufs=1))
        small = ctx.enter_context(tc.tile_pool(name="small", bufs=4))
        upool = ctx.enter_context(tc.tile_pool(name="upool", bufs=2))
        rpool = ctx.enter_context(tc.tile_pool(name="rpool", bufs=2))
        epool = ctx.enter_context(tc.tile_pool(name="epool", bufs=2))
        dpool = ctx.enter_context(tc.tile_pool(name="dpool", bufs=1, space="DRAM"))
        psum_big = ctx.enter_context(tc.tile_pool(name="psb", bufs=2, space="PSUM"))
        psum_av = ctx.enter_context(tc.tile_pool(name="psa", bufs=2, space="PSUM"))

        # ---- constants ----
        xT_sb = consts.tile([128, 4, N_TOK], BF16, tag="xT")
        _load_tiled(nc, xT_sb, xT, 4)
        W_sb = consts.tile([128, 4, 3 * DIM], BF16, tag="W")
        _load_tiled(nc, W_sb, W, 4)
        if trivial:
            cos_sb = consts.tile([128, NT, 64], BF16, tag="cos")
            _load_tiled(nc, cos_sb, T1, NT)
            sinN_sb = consts.tile([128, NT, 64], BF16, tag="sinN")
            _load_tiled(nc, sinN_sb, T2N, NT)
            sinP_sb = consts.tile([128, NT, 64], BF16, tag="sinP")
            _load_tiled(nc, sinP_sb, T2P, NT)
        else:
            T1_sb = consts.tile([128, NT, 1024], BF16, tag="T1")
            _load_tiled(nc, T1_sb, T1, NT)
            T2_sb = consts.tile([128, NT, 1024], BF16, tag="T2")
            _load_tiled(nc, T2_sb, T2, NT)
            T3_sb = consts.tile([128, NT, 1024], BF16, tag="T3")
            _load_tiled(nc, T3_sb, T3, NT)
        eps_sb = consts.tile([128, 1], F32, tag="eps")
        nc.vector.memset(eps_sb, EPS)
        warm = consts.tile([128, 1], F32, tag="warm")
        nc.scalar.activation(warm, eps_sb, mybir.ActivationFunctionType.Exp)

        v_sb = consts.tile([128, NT, NH, VW], BF16, tag="v")
        nc.vector.memset(v_sb[:, :, :, 128:129], 1.0)

        qkT_sb = [
            [
                consts.tile(
                    [128, N_TOK], BF16, name=f"qkT{s}{h}", tag=f"qkT{s}{h}"
                )
                for h in range(NH)
            ]
            for s in range(2)
        ]
        out_sb = consts.tile([128, NT, DIM], F32, tag="osb")
        r_dram = dpool.tile([N_TOK, 2 * DIM], BF16, name="r_dram", tag="r_dram")

        def bcast(ap2d, dims):
            """[128, 64] AP -> [128, *dims, 64] with stride-0 broadcast dims."""
            p, last = ap2d.ap[0], ap2d.ap[-1]
            return bass.AP(
                tensor=ap2d.tensor,
                offset=ap2d.offset,
                ap=[p] + [[0, d] for d in dims] + [last],
            )

        # ---- stage A: qkv, layernorm, rope, transposes ----
        for t in range(NT):
            qkv_ps = psum_big.tile([128, 3 * DIM], F32, tag="big")
            for kc in range(4):
                for j in range(3):
                    nc.tensor.matmul(
                        qkv_ps[:, j * 512 : (j + 1) * 512],
                        lhsT=xT_sb[:, kc, t * 128 : (t + 1) * 128],
                        rhs=W_sb[:, kc, j * 512 : (j + 1) * 512],
                        start=(kc == 0),
                        stop=(kc == 3),
                    )

            # u: normalized q|k, bf16, flat [part, (s h half j) = 1024]
            u = upool.tile([128, 2 * DIM], BF16, tag="u")
            for s in range(2):
                st = small.tile([128, 6], F32, tag=f"st{s}")
                nc.vector.bn_stats(out=st, in_=qkv_ps[:, s * 512 : (s + 1) * 512])
                mv = small.tile([128, 2], F32, tag=f"mv{s}")
                nc.vector.bn_aggr(out=mv, in_=st)
                sd = small.tile([128, 1], F32, tag=f"sd{s}")
                nc.scalar.activation(
                    sd, mv[:, 1:2], mybir.ActivationFunctionType.Sqrt, bias=eps_sb
                )
                rstd = small.tile([128, 1], F32, tag=f"rs{s}")
                nc.vector.reciprocal(rstd, sd)
                nmr = small.tile([128, 1], F32, tag=f"nmr{s}")
                nc.vector.scalar_tensor_tensor(
                    out=nmr,
                    in0=mv[:, 0:1],
                    scalar=-1.0,
                    in1=rstd,
                    op0=mybir.AluOpType.mult,
                    op1=mybir.AluOpType.mult,
                )
                # u = q * rstd + (-mu * rstd), on ScalarE (frees VectorE)
                nc.scalar.activation(
                    out=u[:, s * 512 : (s + 1) * 512],
                    in_=qkv_ps[:, s * 512 : (s + 1) * 512],
                    func=mybir.ActivationFunctionType.Identity,
                    scale=rstd,
                    bias=nmr,
                )

            # v (raw) into augmented per-head layout
            nc.vector.tensor_copy(
                out=v_sb[:, t, :, 0:128],
                in_=v_ps.rearrange("p (h d) -> p h d", h=NH),
            )

            # rope: r = u * T1 + swap_half(u) * T2 (+ T3).
            # half(ap, i): [128, 8 blocks, 64] view selecting half i of each
            # (s, h) 128-wide block -- max 3D free APs for the ISA.
            def half(ap, i):
                return ap.rearrange("p (b half j) -> p b half j", half=2, j=64)[
                    :, :, i, :
                ]

            m1 = rpool.tile([128, 2 * DIM], BF16, tag="m1")
            m2 = rpool.tile([128, 2 * DIM], BF16, tag="m2")
            r = rpool.tile([128, 2 * DIM], BF16, tag="r")
            if trivial:
                nc.vector.tensor_mul(
                    m1.rearrange("p (b j) -> p b j", j=64),
                    u.rearrange("p (b j) -> p b j", j=64),
                    bcast(cos_sb[:, t], (16,)),
                )
                nc.vector.tensor_mul(half(m2, 0), half(u, 1), bcast(sinN_sb[:, t], (8,)))
                nc.vector.tensor_mul(half(m2, 1), half(u, 0), bcast(sinP_sb[:, t], (8,)))
                nc.vector.tensor_add(r, m1, m2)
            else:
                t1v = T1_sb[:, t]
                t2v = T2_sb[:, t]
                t3v = T3_sb[:, t]
                nc.vector.tensor_mul(m1, u, t1v)
                nc.vector.tensor_mul(half(m2, 0), half(u, 1), half(t2v, 0))
                nc.vector.tensor_mul(half(m2, 1), half(u, 0), half(t2v, 1))
                nc.vector.tensor_add(m1, m1, m2)
                nc.vector.tensor_add(r, m1, t3v)

            # spill rope output to DRAM; bulk feature-major transposes later
            nc.sync.dma_start(out=r_dram[t * 128 : (t + 1) * 128, :], in_=r)

            # after each token-half is spilled, transpose that half for all
            # (s, h): [512 tok, 128 feat] -> [128 d, 512 tok].  Lets stage B
            # start before the whole of stage A finishes.
            if t == NT // 2 - 1 or t == NT - 1:
                th = 0 if t == NT // 2 - 1 else 1
                rows = slice(th * 512, (th + 1) * 512)
                for s in range(2):
                    for h in range(NH):
                        blk = (s * NH + h) * 128
                        nc.sync.dma_start(
                            out=qkT_sb[s][h][:, rows],
                            in_=r_dram[rows, blk : blk + 128],
                            transpose=True,
                        )

        # ---- stage B: per-head attention ----
        def emit_scores(h):
            qT, kT = qkT_sb[0][h], qkT_sb[1][h]
            expts = []
            for kc in range(NT):
                sc_ps = psum_big.tile([128, 3 * DIM], F32, tag="big", name="sc_ps")
                for half in range(2):
                    nc.tensor.matmul(
                        sc_ps[:, half * 512 : (half + 1) * 512],
                        lhsT=kT[:, kc * 128 : (kc + 1) * 128],
                        rhs=qT[:, half * 512 : (half + 1) * 512],
                        start=True,
                        stop=True,
                    )
                et = epool.tile([128, N_TOK], BF16, tag=f"exp{kc}", name=f"exp{kc}")
                nc.scalar.activation(
                    out=et,
                    in_=sc_ps[:, 0:N_TOK],
                    func=mybir.ActivationFunctionType.Exp,
                    scale=SCALE,
                )
                expts.append(et)
            return expts

        def emit_av(h, expts):
            for qc in range(NT):
                av = psum_v.tile([128, VW], F32, tag="v", name="av")
                for kc in range(NT):
                    nc.tensor.matmul(
                        av[:, 0:129],
                        lhsT=expts[kc][:, qc * 128 : (qc + 1) * 128],
                        rhs=v_sb[:, kc, h, 0:129],
                        start=(kc == 0),
                        stop=(kc == NT - 1),
                    )
                rcp = small.tile([128, 1], F32, tag="rcp", name="rcp")
                nc.vector.reciprocal(rcp, av[:, 128:129])
                dst = out_sb[:, qc, h * 128 : (h + 1) * 128]
                if h == NH - 1:
                    nc.scalar.activation(
                        dst, av[:, 0:128],
                        mybir.ActivationFunctionType.Copy, scale=rcp,
                    )
                else:
                    nc.vector.tensor_scalar_mul(dst, av[:, 0:128], rcp)
                if h == NH - 1:
                    nc.sync.dma_start(
                        out=out_d.ap()[qc * 128 : (qc + 1) * 128, :],
                        in_=out_sb[:, qc],
                    )

        # interleave: emit scores(h+1) before av(h) so PE fills exp-wait gaps
        exp_cur = emit_scores(0)
        for h in range(NH):
            exp_next = emit_scores(h + 1) if h + 1 < NH else None
            emit_av(h, exp_cur)
            exp_cur = exp_next

    nc.compile()
    return nc


def _rope_tables():
    inv_freq = 1.0 / (10000.0 ** (np.arange(0, D, 2, dtype=np.float32) / D))
    freqs = np.arange(N_TOK, dtype=np.float32)[:, None] * inv_freq[None, :]  # [n, 64]
    return np.cos(freqs), np.sin(freqs)


def _full_tables(g_q, b_q, g_k, b_k):
    """T1/T2/T3 [N_TOK, 1024] with LN gain/bias folded into the rope tables.
    Feature index layout matches u: (s, h, half, j)."""
    cos64, sin64 = _rope_tables()
    T1 = np.empty((N_TOK, 1024), np.float32)
    T2 = np.empty((N_TOK, 1024), np.float32)
    T3 = np.empty((N_TOK, 1024), np.float32)
    for s, (g, b) in enumerate(((g_q, b_q), (g_k, b_k))):
        g = g.reshape(NH, 2, 64)
        b = b.reshape(NH, 2, 64)
        for h in range(NH):
            base = s * 512 + h * 128
            lo, hi = slice(base, base + 64), slice(base + 64, base + 128)
            T1[:, lo] = g[h, 0] * cos64
            T1[:, hi] = g[h, 1] * cos64
            T2[:, lo] = -g[h, 1] * sin64
            T2[:, hi] = g[h, 0] * sin64
            T3[:, lo] = b[h, 0] * cos64 - b[h, 1] * sin64
            T3[:, hi] = b[h, 1] * cos64 + b[h, 0] * sin64
    return T1, T2, T3


def make_in_maps(x, a, Wqkv_x, Wqkv_a, g_qx, b_qx, g_kx, b_kx, g_qa, b_qa, g_ka, b_ka):
    """Returns (trivial, in_maps) for the 8 cores: core c = (batch c//2, modality c%2)."""
    x, a = np.asarray(x), np.asarray(a)
    Ws = (np.asarray(Wqkv_x), np.asarray(Wqkv_a))
    gb = (
        (np.asarray(g_qx), np.asarray(b_qx), np.asarray(g_kx), np.asarray(b_kx)),
        (np.asarray(g_qa), np.asarray(b_qa), np.asarray(g_ka), np.asarray(b_ka)),
    )
    trivial = all(
        np.all(g == 1.0) and np.all(b == 0.0)
        for (gq, bq, gk, bk) in gb
        for g, b in ((gq, bq), (gk, bk))
    )
    cos64, sin64 = _rope_tables()
    in_maps = []
    for c in range(8):
        i, m = c // 2, c % 2
        src = x[i] if m == 0 else a[i]
        im = {
            "xT": np.ascontiguousarray(src.T).astype(NPBF16),
            "W": Ws[m].astype(NPBF16),
        }
        if trivial:
            im["T1"] = cos64.astype(NPBF16)
            im["T2N"] = (-sin64).astype(NPBF16)
            im["T2P"] = sin64.astype(NPBF16)
        else:
            gq, bq, gk, bk = gb[m]
            T1, T2, T3 = _full_tables(gq, bq, gk, bk)
            im["T1"] = T1.astype(NPBF16)
            im["T2"] = T2.astype(NPBF16)
            im["T3"] = T3.astype(NPBF16)
        in_maps.append(im)
    return trivial, in_maps


_module_cache: dict[bool, object] = {}


def _get_module(trivial: bool):
    if trivial not in _module_cache:
        _module_cache[trivial] = build_module(trivial)
    return _module_cache[trivial]


def kernel(**inputs) -> np.ndarray:
    trivial, in_maps = make_in_maps(**inputs)
    nc = _get_module(trivial)
    res = run_bass_kernel_spmd(nc, in_maps, core_ids=list(range(8)))
    out = np.empty((4, N_TOK, 2 * DIM), np.float32)
    for c in range(8):
        i, m = c // 2, c % 2
        out[i, :, m * 512 : (m + 1) * 512] = res.results[c]["out"]
    return out


# revision 16
# speedup vs baseline: 1.4074x; 1.0059x over previous
"""Cross-modal attention (CMAttention) Trainium2 kernel.

Strategy: 8-way SPMD over (batch=4) x (modality=2). After the reference's
concat([q_x, q_a]) and 8-head split with head_dim=128, heads 0-3 depend only
on modality x and heads 4-7 only on modality a.  Each core therefore owns one
(batch, modality) pair end-to-end with zero communication:
  QKV projection (bf16 matmul) -> LayerNorm on q,k (bn_stats) -> RoPE
  (table multiplies) -> per-head DMA-transpose of q,k to [d, tok] ->
  scores^T matmul -> exp on ScalarE (scale folded) -> attn @ [v | 1]
  (ones column yields the softmax denominator for free) -> normalize.
"""

import os
import sys

for _p in ("/opt/trn_rl_repo", os.path.expanduser("~/.axon_site/_ro/trn_rl_repo")):
    if os.path.isdir(_p) and _p not in sys.path:
        sys.path.append(_p)

from contextlib import ExitStack

import ml_dtypes
import numpy as np

import concourse.bacc as bacc
import concourse.bass as bass
import concourse.mybir as mybir
import concourse.tile as tile
from concourse.bass_utils import run_bass_kernel_spmd

if os.environ.get("K_LDWOPT"):
    import concourse.bass_utils as _bu

    _orig_run_command = _bu.run_command

    def _patched_run_command(argv, **kw):
        argv = [
            "--enable-ldw-opt=true" if a == "--enable-ldw-opt=false" else a
            for a in argv
        ]
        return _orig_run_command(argv, **kw)

    _bu.run_command = _patched_run_command

BF16 = mybir.dt.bfloat16
F32 = mybir.dt.float32
NPBF16 = ml_dtypes.bfloat16

DIM = 512          # per-modality feature dim
N_TOK = 1024       # sequence length
NH = 4             # heads handled per core (one modality's heads)
D = 128            # head dim
NT = 8             # token tiles of 128
EPS = 1e-5
SCALE = 1.0 / float(np.sqrt(D))
VW = 132           # per-head v block width: 128 d + 1 ones + 3 pad


def _load_tiled(nc, sbuf_tile, dram, blocks):
    """DMA a [blocks*128, C] DRAM tensor into a [128, blocks, C] SBUF tile.
    GpSimd's SWDGE ring is idle at kernel start; using it keeps the sync
    ring free for the xT/W loads that gate the first matmul."""
    nc.gpsimd.dma_start(
        out=sbuf_tile, in_=dram.ap().rearrange("(a b) c -> b a c", b=128)
    )


def build_module(trivial: bool):
    """Build the per-core Bass program.  trivial=True assumes all LN gains are
    exactly 1 and biases exactly 0 (folded tables are plain cos/sin and the
    additive rope term vanishes); trivial=False uses full-width tables with
    g folded in and an extra additive T3 table."""
    nc = bacc.Bacc("TRN2", target_bir_lowering=False, debug=False, num_devices=8)

    xT = nc.dram_tensor("xT", [DIM, N_TOK], BF16, kind="ExternalInput")
    W = nc.dram_tensor("W", [DIM, 3 * DIM], BF16, kind="ExternalInput")
    if trivial:
        T1 = nc.dram_tensor("T1", [N_TOK, 64], BF16, kind="ExternalInput")
        T2N = nc.dram_tensor("T2N", [N_TOK, 64], BF16, kind="ExternalInput")
        T2P = nc.dram_tensor("T2P", [N_TOK, 64], BF16, kind="ExternalInput")
    else:
        T1 = nc.dram_tensor("T1", [N_TOK, 1024], BF16, kind="ExternalInput")
        T2 = nc.dram_tensor("T2", [N_TOK, 1024], BF16, kind="ExternalInput")
        T3 = nc.dram_tensor("T3", [N_TOK, 1024], BF16, kind="ExternalInput")
    out_d = nc.dram_tensor("out", [N_TOK, DIM], F32, kind="ExternalOutput")

    with tile.TileContext(nc) as tc, ExitStack() as ctx:
        consts = ctx.enter_context(tc.tile_pool(name="consts", bufs=1))
        small = ctx.enter_context(tc.tile_pool(name="small", bufs=4))
        upool = ctx.enter_context(tc.tile_pool(name="upool", bufs=2))
        rpool = ctx.enter_context(tc.tile_pool(name="rpool", bufs=2))
        epool = ctx.enter_context(tc.tile_pool(name="epool", bufs=2))
        dpool = ctx.enter_context(tc.tile_pool(name="dpool", bufs=1, space="DRAM"))
        psum_big = ctx.enter_context(tc.tile_pool(name="psb", bufs=2, space="PSUM"))
        psum_av = ctx.enter_context(tc.tile_pool(name="psa", bufs=2, space="PSUM"))

        # ---- constants ----
        xT_sb = consts.tile([128, 4, N_TOK], BF16, tag="xT")
        _load_tiled(nc, xT_sb, xT, 4)
        W_sb = consts.tile([128, 4, 3 * DIM], BF16, tag="W")
        _load_tiled(nc, W_sb, W, 4)
        if trivial:
            cos_sb = consts.tile([128, NT, 64], BF16, tag="cos")
            _load_tiled(nc, cos_sb, T1, NT)
            sinN_sb = consts.tile([128, NT, 64], BF16, tag="sinN")
            _load_tiled(nc, sinN_sb, T2N, NT)
            sinP_sb = consts.tile([128, NT, 64], BF16, tag="sinP")
            _load_tiled(nc, sinP_sb, T2P, NT)
        else:
            T1_sb = consts.tile([128, NT, 1024], BF16, tag="T1")
            _load_tiled(nc, T1_sb, T1, NT)
            T2_sb = consts.tile([128, NT, 1024], BF16, tag="T2")
            _load_tiled(nc, T2_sb, T2, NT)
            T3_sb = consts.tile([128, NT, 1024], BF16, tag="T3")
            _load_tiled(nc, T3_sb, T3, NT)
        eps_sb = consts.tile([128, 1], F32, tag="eps")
        nc.vector.memset(eps_sb, EPS)
        warm = consts.tile([128, 1], F32, tag="warm")
        nc.scalar.activation(warm, eps_sb, mybir.ActivationFunctionType.Exp)

        v_sb = consts.tile([128, NT, NH, VW], BF16, tag="v")
        nc.vector.memset(v_sb[:, :, :, 128:129], 1.0)

        qkT_sb = [
            [
                consts.tile(
                    [128, N_TOK], BF16, name=f"qkT{s}{h}", tag=f"qkT{s}{h}"
                )
                for h in range(NH)
            ]
            for s in range(2)
        ]
        out_sb = consts.tile([128, NT, DIM], F32, tag="osb")
        r_dram = dpool.tile([N_TOK, 2 * DIM], BF16, name="r_dram", tag="r_dram")

        def bcast(ap2d, dims):
            """[128, 64] AP -> [128, *dims, 64] with stride-0 broadcast dims."""
            p, last = ap2d.ap[0], ap2d.ap[-1]
            return bass.AP(
                tensor=ap2d.tensor,
                offset=ap2d.offset,
                ap=[p] + [[0, d] for d in dims] + [last],
            )

        # ---- stage A: qkv, layernorm, rope, transposes ----
        for t in range(NT):
            qkv_ps = psum_big.tile([128, 3 * DIM], F32, tag="big")
            for kc in range(4):
                for j in range(3):
                    nc.tensor.matmul(
                        qkv_ps[:, j * 512 : (j + 1) * 512],
                        lhsT=xT_k[kc][:, t * 128 : (t + 1) * 128],
                        rhs=W_k[kc][:, j * 512 : (j + 1) * 512],
                        start=(kc == 0),
                        stop=(kc == 3),
                    )

            # u: normalized q|k, bf16, flat [part, (s h half j) = 1024]
            u = upool.tile([128, 2 * DIM], BF16, tag="u")
            for s in range(2):
                st = small.tile([128, 6], F32, tag=f"st{s}")
                nc.vector.bn_stats(out=st, in_=qkv_ps[:, s * 512 : (s + 1) * 512])
                mv = small.tile([128, 2], F32, tag=f"mv{s}")
                nc.vector.bn_aggr(out=mv, in_=st)
                sd = small.tile([128, 1], F32, tag=f"sd{s}")
                nc.scalar.activation(
                    sd, mv[:, 1:2], mybir.ActivationFunctionType.Sqrt, bias=eps_sb
                )
                rstd = small.tile([128, 1], F32, tag=f"rs{s}")
                nc.vector.reciprocal(rstd, sd)
                nmr = small.tile([128, 1], F32, tag=f"nmr{s}")
                nc.vector.scalar_tensor_tensor(
                    out=nmr,
                    in0=mv[:, 0:1],
                    scalar=-1.0,
                    in1=rstd,
                    op0=mybir.AluOpType.mult,
                    op1=mybir.AluOpType.mult,
                )
                # u = q * rstd + (-mu * rstd), on ScalarE (frees VectorE)
                nc.scalar.activation(
                    out=u[:, s * 512 : (s + 1) * 512],
                    in_=qkv_ps[:, s * 512 : (s + 1) * 512],
                    func=mybir.ActivationFunctionType.Identity,
                    scale=rstd,
                    bias=nmr,
                )

            # v (raw) into augmented per-head layout
            nc.vector.tensor_copy(
                out=v_sb[:, t, :, 0:128],
                in_=v_ps.rearrange("p (h d) -> p h d", h=NH),
            )

            # rope: r = u * T1 + swap_half(u) * T2 (+ T3).
            # half(ap, i): [128, 8 blocks, 64] view selecting half i of each
            # (s, h) 128-wide block -- max 3D free APs for the ISA.
            def half(ap, i):
                return ap.rearrange("p (b half j) -> p b half j", half=2, j=64)[
                    :, :, i, :
                ]

            m1 = rpool.tile([128, 2 * DIM], BF16, tag="m1")
            m2 = rpool.tile([128, 2 * DIM], BF16, tag="m2")
            r = rpool.tile([128, 2 * DIM], BF16, tag="r")
            if trivial:
                nc.vector.tensor_mul(
                    m1.rearrange("p (b j) -> p b j", j=64),
                    u.rearrange("p (b j) -> p b j", j=64),
                    bcast(cos_sb[:, t], (16,)),
                )
                nc.vector.tensor_mul(half(m2, 0), half(u, 1), bcast(sinN_sb[:, t], (8,)))
                nc.vector.tensor_mul(half(m2, 1), half(u, 0), bcast(sinP_sb[:, t], (8,)))
                nc.vector.tensor_add(r, m1, m2)
            else:
                t1v = T1_sb[:, t]
                t2v = T2_sb[:, t]
                t3v = T3_sb[:, t]
                nc.vector.tensor_mul(m1, u, t1v)
                nc.vector.tensor_mul(half(m2, 0), half(u, 1), half(t2v, 0))
                nc.vector.tensor_mul(half(m2, 1), half(u, 0), half(t2v, 1))
                nc.vector.tensor_add(m1, m1, m2)
                nc.vector.tensor_add(r, m1, t3v)

            # spill rope output to DRAM; bulk feature-major transposes later
            nc.sync.dma_start(out=r_dram[t * 128 : (t + 1) * 128, :], in_=r)

            # after each token-half is spilled, transpose that half for all
            # (s, h): [512 tok, 128 feat] -> [128 d, 512 tok].  Lets stage B
            # start before the whole of stage A finishes.
            if t == NT // 2 - 1 or t == NT - 1:
                th = 0 if t == NT // 2 - 1 else 1
                rows = slice(th * 512, (th + 1) * 512)
                for s in range(2):
                    for h in range(NH):
                        blk = (s * NH + h) * 128
                        nc.sync.dma_start(
                            out=qkT_sb[s][h][:, rows],
                            in_=r_dram[rows, blk : blk + 128],
                            transpose=True,
                        )

        # ---- stage B: per-head attention ----
        def emit_scores(h):
            qT, kT = qkT_sb[0][h], qkT_sb[1][h]
            expts = []
            for kc in range(NT):
                sc_ps = psum_big.tile([128, 3 * DIM], F32, tag="big", name="sc_ps")
                for half in range(2):
                    nc.tensor.matmul(
                        sc_ps[:, half * 512 : (half + 1) * 512],
                        lhsT=kT[:, kc * 128 : (kc + 1) * 128],
                        rhs=qT[:, half * 512 : (half + 1) * 512],
                        start=True,
                        stop=True,
                    )
                et = epool.tile([128, N_TOK], BF16, tag=f"exp{kc}", name=f"exp{kc}")
                nc.scalar.activation(
                    out=et,
                    in_=sc_ps[:, 0:N_TOK],
                    func=mybir.ActivationFunctionType.Exp,
                    scale=SCALE,
                )
                expts.append(et)
            return expts

        def emit_av(h, expts):
            for qc in range(NT):
                av = psum_v.tile([128, VW], F32, tag="v", name="av")
                for kc in range(NT):
                    nc.tensor.matmul(
                        av[:, 0:129],
                        lhsT=expts[kc][:, qc * 128 : (qc + 1) * 128],
                        rhs=v_sb[:, kc, h, 0:129],
                        start=(kc == 0),
                        stop=(kc == NT - 1),
                    )
                rcp = small.tile([128, 1], F32, tag="rcp", name="rcp")
                nc.vector.reciprocal(rcp, av[:, 128:129])
                dst = out_sb[:, qc, h * 128 : (h + 1) * 128]
                if h == NH - 1:
                    nc.scalar.activation(
                        dst, av[:, 0:128],
                        mybir.ActivationFunctionType.Copy, scale=rcp,
                    )
                else:
                    nc.vector.tensor_scalar_mul(dst, av[:, 0:128], rcp)
                if h == NH - 1:
                    nc.sync.dma_start(
                        out=out_d.ap()[qc * 128 : (qc + 1) * 128, :],
                        in_=out_sb[:, qc],
                    )

        # interleave: emit scores(h+1) before av(h) so PE fills exp-wait gaps
        exp_cur = emit_scores(0)
        for h in range(NH):
            exp_next = emit_scores(h + 1) if h + 1 < NH else None
            emit_av(h, exp_cur)
            exp_cur = exp_next

    nc.compile()
    return nc


def _rope_tables():
    inv_freq = 1.0 / (10000.0 ** (np.arange(0, D, 2, dtype=np.float32) / D))
    freqs = np.arange(N_TOK, dtype=np.float32)[:, None] * inv_freq[None, :]  # [n, 64]
    return np.cos(freqs), np.sin(freqs)


def _full_tables(g_q, b_q, g_k, b_k):
    """T1/T2/T3 [N_TOK, 1024] with LN gain/bias folded into the rope tables.
    Feature index layout matches u: (s, h, half, j)."""
    cos64, sin64 = _rope_tables()
    T1 = np.empty((N_TOK, 1024), np.float32)
    T2 = np.empty((N_TOK, 1024), np.float32)
    T3 = np.empty((N_TOK, 1024), np.float32)
    for s, (g, b) in enumerate(((g_q, b_q), (g_k, b_k))):
        g = g.reshape(NH, 2, 64)
        b = b.reshape(NH, 2, 64)
        for h in range(NH):
            base = s * 512 + h * 128
            lo, hi = slice(base, base + 64), slice(base + 64, base + 128)
            T1[:, lo] = g[h, 0] * cos64
            T1[:, hi] = g[h, 1] * cos64
            T2[:, lo] = -g[h, 1] * sin64
            T2[:, hi] = g[h, 0] * sin64
            T3[:, lo] = b[h, 0] * cos64 - b[h, 1] * sin64
            T3[:, hi] = b[h, 1] * cos64 + b[h, 0] * sin64
    return T1, T2, T3


def make_in_maps(x, a, Wqkv_x, Wqkv_a, g_qx, b_qx, g_kx, b_kx, g_qa, b_qa, g_ka, b_ka):
    """Returns (trivial, in_maps) for the 8 cores: core c = (batch c//2, modality c%2)."""
    x, a = np.asarray(x), np.asarray(a)
    Ws = (np.asarray(Wqkv_x), np.asarray(Wqkv_a))
    gb = (
        (np.asarray(g_qx), np.asarray(b_qx), np.asarray(g_kx), np.asarray(b_kx)),
        (np.asarray(g_qa), np.asarray(b_qa), np.asarray(g_ka), np.asarray(b_ka)),
    )
    trivial = all(
        np.all(g == 1.0) and np.all(b == 0.0)
        for (gq, bq, gk, bk) in gb
        for g, b in ((gq, bq), (gk, bk))
    )
    cos64, sin64 = _rope_tables()
    in_maps = []
    for c in range(8):
        i, m = c // 2, c % 2
        src = x[i] if m == 0 else a[i]
        im = {
            "xT": np.ascontiguousarray(src.T).astype(NPBF16),
            "W": Ws[m].astype(NPBF16),
        }
        if trivial:
            im["T1"] = cos64.astype(NPBF16)
            im["T2N"] = (-sin64).astype(NPBF16)
            im["T2P"] = sin64.astype(NPBF16)
        else:
            gq, bq, gk, bk = gb[m]
            T1, T2, T3 = _full_tables(gq, bq, gk, bk)
            im["T1"] = T1.astype(NPBF16)
            im["T2"] = T2.astype(NPBF16)
            im["T3"] = T3.astype(NPBF16)
        in_maps.append(im)
    return trivial, in_maps


_module_cache: dict[bool, object] = {}


def _get_module(trivial: bool):
    if trivial not in _module_cache:
        _module_cache[trivial] = build_module(trivial)
    return _module_cache[trivial]


def kernel(**inputs) -> np.ndarray:
    trivial, in_maps = make_in_maps(**inputs)
    nc = _get_module(trivial)
    res = run_bass_kernel_spmd(nc, in_maps, core_ids=list(range(8)))
    out = np.empty((4, N_TOK, 2 * DIM), np.float32)
    for c in range(8):
        i, m = c // 2, c % 2
        out[i, :, m * 512 : (m + 1) * 512] = res.results[c]["out"]
    return out
